# revision 13
# baseline (speedup 1.0000x reference)
"""Debiased EMA (nn_DebiasedEMA) Trainium2 Bass kernel.

x: [B=32, T=4096, C=512] f32.
    y_t = a*y_{t-1} + (1-a)*x_t  (y_0 = x_0), a = f32(0.9)
    out_t = y_t / max(1 - a^(t+1), 1e-6)

Formulation (exact, no window truncation): split T into 127-row blocks in
the natural [T, C] layout.  For block i with local row m (t = 127i + m),

    y_t = sum_{k=0..126} (1-a) a^(m-k) [k<=m] * x_block_i[k]
        + a^(m+1) * carry_{i-1}

with carry_{i-1} = y_{127i-1}, the last EMA row of the previous block.
Both terms fold into a SINGLE K=128 fp32 matmul per block: the rhs tile
holds the 127 current x rows on partitions 0..126 and the carry vector on
partition 127; the lhsT weight matrix holds the triangular decay profile
in rows 0..126 and v[m] = a^(m+1) in row 127.  PE cost: one fp32 matmul
(~0.85us) per 127 output rows — half of the sliding-window formulation.

The debias divisor is exactly 1.0 in fp32 for t >= 165, so it is folded
into the block-0/1 weight matrices (block 0 also carries the y_0 = x_0
initial condition in its k=0 column, and zero in the carry row); block 1's
carry row also undoes block 0's row-126 debias.

The carry chain is a 2KB SBUF->SBUF DMA per block: output row 126 (copied
out of PSUM first, as a tiny row copy) -> partition 127 of the next
block's rhs slice.  The 4 sequences per core are interleaved round-robin,
so each chain step has ~4 matmuls of slack.

T = 4096 = 32*127 + 32: the final partial block uses a K=33 matmul
(32 x rows + carry on partition 32).

Sharding: batch-parallel, 4 sequences per NeuronCore, no communication.
"""

import sys

for _p in ("/opt/trn_rl_repo", "/opt/pypackages"):
    if _p not in sys.path:
        sys.path.insert(0, _p)

import numpy as np

import concourse.bacc as bacc
import concourse.mybir as mybir
from concourse import bass_utils
from concourse.tile import TileContext

B, T, C = 32, 4096, 512
NCORES = 8
BPC = B // NCORES        # sequences per core
L = 127                  # time-block length (partition 127 carries state)
ALPHA = 0.9
DENOM_MIN = 1e-6

F32 = mybir.dt.float32


def _plan(t_len: int):
    """Number of full 127-row blocks and the size of the final partial."""
    nfull = t_len // L
    rem = t_len - nfull * L
    return nfull, rem


def _build_weights(t_len: int = T) -> np.ndarray:
    """lhsT weight matrices, packed [128, 3*127 + rem] f32.

    Columns: [W0 (block 0) | W1 (block 1) | W (blocks >= 2) | Wlast (rem)].
    Each W* is [128, 127]: rows k=0..126 hold the triangular decay
    (1-a)a^(m-k) (block 0: a^m in the k=0 column instead), row k=127 holds
    the carry weight a^(m+1) (block 0: zero).  Debias folds: block 0 rows
    /d[m], block 1 rows /d[127+m] with carry row also * d[126].
    Wlast is [33, rem] padded to [128, rem]: 32 x rows + carry row at k=32.
    Built in float64 from the f32-rounded alpha, rounded once to f32.
    """
    a = float(np.float32(ALPHA))     # f32 value of clip(0.9) as f64
    omb = 1.0 - a                    # exact (Sterbenz), matches f32 1-a
    nfull, rem = _plan(t_len)
    t = np.arange(2 * L + 2, dtype=np.float64)
    d = np.maximum(1.0 - a ** (t + 1.0), DENOM_MIN)

    k = np.arange(L, dtype=np.float64)[:, None]   # input row 0..126
    m = np.arange(L, dtype=np.float64)[None, :]   # output row 0..126
    tri = (m - k) >= 0
    dec = np.where(tri, omb * a ** np.where(tri, m - k, 0.0), 0.0)
    vrow = a ** (m + 1.0)                          # carry weight, [1, 127]

    # block 0: x0 column is a^m (not (1-a)a^m); no carry; rows / d[m]
    W0 = np.where((k == 0) & tri, a**m, dec)
    W0 = np.concatenate([W0, np.zeros((1, L))], axis=0) / d[:L][None, :]
    # block 1: rows / d[127+m]; carry row undoes block 0's /d[126]
    W1 = np.concatenate([dec, vrow * d[L - 1]], axis=0) / d[L:2 * L][None, :]
    # blocks >= 2: debias is exactly 1.0 in f32
    W = np.concatenate([dec, vrow], axis=0)
    # final partial block (rem rows): K = rem + 1, padded to 128 partitions
    kl = np.arange(rem, dtype=np.float64)[:, None]
    ml = np.arange(rem, dtype=np.float64)[None, :]
    tril = (ml - kl) >= 0
    decl = np.where(tril, omb * a ** np.where(tril, ml - kl, 0.0), 0.0)
    Wlast = np.concatenate(
        [decl, a ** (ml + 1.0), np.zeros((L - rem, rem))], axis=0)

    w = np.concatenate([W0, W1, W, Wlast], axis=1)
    assert w.shape == (128, 3 * L + rem)
    return np.ascontiguousarray(w.astype(np.float32))


def build_program(bpc: int = BPC, t_len: int = T, chunk: int = 4):
    """One core's program: EMA over `bpc` independent [t_len, C] sequences,
    interleaved round-robin so each carry chain has `bpc` blocks of slack."""
    nfull, rem = _plan(t_len)
    nchunk = nfull // chunk
    assert nchunk * chunk == nfull and rem > 0
    assert bpc <= 4

    nc = bacc.Bacc("TRN2", target_bir_lowering=False, debug=False)
    x = nc.dram_tensor("x", [bpc * t_len, C], F32, kind="ExternalInput").ap()
    w = nc.dram_tensor("w", [128, 3 * L + rem], F32,
                       kind="ExternalInput").ap()
    y = nc.dram_tensor("y", [bpc * t_len, C], F32, kind="ExternalOutput").ap()

    with TileContext(nc) as tc:
        with (
            tc.tile_pool(name="wpool", bufs=1) as wpool,
            tc.tile_pool(name="xpool", bufs=2 * bpc + 1) as xpool,
            tc.tile_pool(name="ypool", bufs=2 * bpc + 1) as ypool,
            tc.tile_pool(name="psum", bufs=8, space="PSUM") as ppool,
        ):
            wt = wpool.tile([128, 3 * L + rem], F32)
            nc.sync.dma_start(out=wt[:, :], in_=w[:, :])
            W0w = wt[:, 0 * L:1 * L]
            W1w = wt[:, 1 * L:2 * L]
            Ww = wt[:, 2 * L:3 * L]
            Wlast = wt[0:rem + 1, 3 * L:3 * L + rem]

            nblk = nfull + 1          # per-sequence blocks incl. partial
            xt: dict = {}
            yt: dict = {}
            # (dram_row, width, tile, free_off, carry_part) per (b, block)
            eng_i = 0

            def alloc_chunk(ch):
                """Allocate + DMA the x tiles for chunk `ch` of each batch,
                returning {b: tile}.  Chunk ch covers blocks
                ch*chunk..ch*chunk+chunk-1 (127 rows each) or the final
                partial block when ch == nchunk."""
                tiles = {}
                for b in range(bpc):
                    if ch < nchunk:
                        r0 = b * t_len + ch * chunk * L
                        tl = xpool.tile([128, chunk * C], F32, tag="xt",
                                        name=f"xt_{ch}_{b}")
                        if ch == 0:
                            # block 0 has no carry: zero its state slot
                            # (partition 127; compute ops must start 32-
                            # aligned, so clear [96:128) BEFORE the x DMA
                            # overwrites rows 96..126 with real data)
                            nc.gpsimd.memset(tl[96:128, 0:C], 0.0)
                            nc.sync.dma_start(out=tl[0:L, 0:C],
                                              in_=x[r0:r0 + L, :])
                            if chunk > 1:
                                nc.sync.dma_start(
                                    out=tl[0:L, C:].rearrange(
                                        "p (n c) -> p n c", c=C),
                                    in_=x[r0 + L:r0 + chunk * L, :].rearrange(
                                        "(n p) c -> p n c", p=L),
                                )
                        else:
                            nc.sync.dma_start(
                                out=tl[0:L, :].rearrange(
                                    "p (n c) -> p n c", c=C),
                                in_=x[r0:r0 + chunk * L, :].rearrange(
                                    "(n p) c -> p n c", p=L),
                            )
                    else:
                        r0 = b * t_len + nfull * L
                        tl = xpool.tile([128, C], F32, tag="xt",
                                        name=f"xt_last_{b}")
                        nc.sync.dma_start(out=tl[0:rem, :],
                                          in_=x[r0:r0 + rem, :])
                    tiles[b] = tl
                return tiles

            xt = alloc_chunk(0)
            for ch in range(nchunk + 1):
                last_chunk = ch == nchunk
                blocks = 1 if last_chunk else chunk
                for b in range(bpc):
                    yt[b] = ypool.tile(
                        [128, C if last_chunk else chunk * C], F32,
                        tag="yt", name=f"yt_{ch}_{b}")
                nxt = None
                for j in range(blocks):
                    r = ch * chunk + j
                    if j == 0 and not last_chunk:
                        nxt = alloc_chunk(ch + 1)   # prefetch next chunk
                    ps = {}
                    for b in range(bpc):
                        if last_chunk:
                            ps[b] = ppool.tile([rem, C], F32, tag="ps",
                                               name=f"ps_{ch}_{j}_{b}")
                            nc.tensor.matmul(ps[b][:, :], Wlast,
                                             xt[b][0:rem + 1, 0:C],
                                             start=True, stop=True)
                        else:
                            ps[b] = ppool.tile([L, C], F32, tag="ps",
                                               name=f"ps_{ch}_{j}_{b}")
                            cw = W0w if r == 0 else (W1w if r == 1 else Ww)
                            nc.tensor.matmul(ps[b][:, :], cw,
                                             xt[b][:, j * C:(j + 1) * C],
                                             start=True, stop=True)
                    for b in range(bpc):
                        rows = rem if last_chunk else L
                        if r < nblk - 1:
                            # high rows (96..126, incl. the carry row 126)
                            # first, as a small 32-aligned copy, so the
                            # carry DMA never waits on the full-tile copy
                            nc.vector.tensor_copy(
                                out=yt[b][96:L, j * C:(j + 1) * C],
                                in_=ps[b][96:L, :])
                            # carry -> state slot of the next block's rhs
                            if j + 1 < blocks:
                                tgt = xt[b][L:L + 1, (j + 1) * C:(j + 2) * C]
                            elif ch + 1 == nchunk:
                                tgt = nxt[b][rem:rem + 1, 0:C]
                            else:
                                tgt = nxt[b][L:L + 1, 0:C]
                            nc.sync.dma_start(
                                out=tgt,
                                in_=yt[b][L - 1:L, j * C:(j + 1) * C])
                            dst = yt[b][0:96, j * C:(j + 1) * C]
                            src = ps[b][0:96, :]
                        else:
                            dst = yt[b][0:rows, j * C:(j + 1) * C]
                            src = ps[b][0:rows, :]
                        if eng_i % 2 == 0:
                            nc.vector.tensor_copy(out=dst, in_=src)
                        else:
                            nc.scalar.copy(dst, src)
                        eng_i += 1
                if nxt is not None:
                    xt = nxt
                for b in range(bpc):
                    if last_chunk:
                        r0 = b * t_len + nfull * L
                        nc.gpsimd.dma_start(out=y[r0:r0 + rem, :],
                                            in_=yt[b][0:rem, :])
                    else:
                        r0 = b * t_len + ch * chunk * L
                        nc.gpsimd.dma_start(
                            out=y[r0:r0 + chunk * L, :].rearrange(
                                "(n p) c -> p n c", p=L),
                            in_=yt[b][0:L, :].rearrange(
                                "p (n c) -> p n c", c=C),
                        )
    nc.compile()
    return nc


_CACHE: dict = {}


def _get_program():
    if "nc" not in _CACHE:
        _CACHE["nc"] = build_program()
        _CACHE["w"] = _build_weights()
    return _CACHE["nc"], _CACHE["w"]


def _run(x: np.ndarray, trace: bool = False):
    nc, w = _get_program()
    in_maps = [
        {
            "x": np.ascontiguousarray(
                x[k * BPC:(k + 1) * BPC].reshape(BPC * T, C)),
            "w": w,
        }
        for k in range(NCORES)
    ]
    res = bass_utils.run_bass_kernel_spmd(
        nc, in_maps, core_ids=list(range(NCORES)), trace=trace)
    y = np.concatenate(
        [r["y"].reshape(BPC, T, C) for r in res.results], axis=0)
    return y, res


def kernel(x) -> np.ndarray:
    x = np.asarray(x, dtype=np.float32)
    assert x.shape == (B, T, C), x.shape
    y, _ = _run(x, trace=False)
    return y


# revision 14
# speedup vs baseline: 1.0445x; 1.0445x over previous
"""Debiased EMA (nn_DebiasedEMA) Trainium2 Bass kernel.

x: [B=32, T=4096, C=512] f32.
    y_t = a*y_{t-1} + (1-a)*x_t  (y_0 = x_0), a = f32(0.9)
    out_t = y_t / max(1 - a^(t+1), 1e-6)

Formulation (exact, no window truncation): split T into 127-row blocks in
the natural [T, C] layout.  For block i with local row m (t = 127i + m),

    y_t = sum_{k=0..126} (1-a) a^(m-k) [k<=m] * x_block_i[k]
        + a^(m+1) * carry_{i-1}

with carry_{i-1} = y_{127i-1}, the last EMA row of the previous block.
Both terms fold into a SINGLE K=128 fp32 matmul per block: the rhs tile
holds the 127 current x rows on partitions 0..126 and the carry vector on
partition 127; the lhsT weight matrix holds the triangular decay profile
in rows 0..126 and v[m] = a^(m+1) in row 127.  PE cost: one fp32 matmul
(~0.85us) per 127 output rows — half of the sliding-window formulation.

The debias divisor is exactly 1.0 in fp32 for t >= 165, so it is folded
into the block-0/1 weight matrices (block 0 also carries the y_0 = x_0
initial condition in its k=0 column, and zero in the carry row); block 1's
carry row also undoes block 0's row-126 debias.

The carry chain is a 2KB SBUF->SBUF DMA per block: output row 126 (copied
out of PSUM first, as a tiny row copy) -> partition 127 of the next
block's rhs slice.  The 4 sequences per core are interleaved round-robin,
so each chain step has ~4 matmuls of slack.

T = 4096 = 32*127 + 32: the final partial block uses a K=33 matmul
(32 x rows + carry on partition 32).

Sharding: batch-parallel, 4 sequences per NeuronCore, no communication.
"""

import sys

for _p in ("/opt/trn_rl_repo", "/opt/pypackages"):
    if _p not in sys.path:
        sys.path.insert(0, _p)

import numpy as np

import concourse.bacc as bacc
import concourse.mybir as mybir
from concourse import bass_utils
from concourse.tile import TileContext

B, T, C = 32, 4096, 512
NCORES = 8
BPC = B // NCORES        # sequences per core
L = 127                  # time-block length (partition 127 carries state)
ALPHA = 0.9
DENOM_MIN = 1e-6

F32 = mybir.dt.float32


def _plan(t_len: int):
    """Number of full 127-row blocks and the size of the final partial."""
    nfull = t_len // L
    rem = t_len - nfull * L
    return nfull, rem


def _build_weights(t_len: int = T) -> np.ndarray:
    """lhsT weight matrices, packed [128, 3*127 + rem] f32.

    Columns: [W0 (block 0) | W1 (block 1) | W (blocks >= 2) | Wlast (rem)].
    Each W* is [128, 127]: rows k=0..126 hold the triangular decay
    (1-a)a^(m-k) (block 0: a^m in the k=0 column instead), row k=127 holds
    the carry weight a^(m+1) (block 0: zero).  Debias folds: block 0 rows
    /d[m], block 1 rows /d[127+m] with carry row also * d[126].
    Wlast is [33, rem] padded to [128, rem]: 32 x rows + carry row at k=32.
    Built in float64 from the f32-rounded alpha, rounded once to f32.
    """
    a = float(np.float32(ALPHA))     # f32 value of clip(0.9) as f64
    omb = 1.0 - a                    # exact (Sterbenz), matches f32 1-a
    nfull, rem = _plan(t_len)
    t = np.arange(2 * L + 2, dtype=np.float64)
    d = np.maximum(1.0 - a ** (t + 1.0), DENOM_MIN)

    k = np.arange(L, dtype=np.float64)[:, None]   # input row 0..126
    m = np.arange(L, dtype=np.float64)[None, :]   # output row 0..126
    tri = (m - k) >= 0
    dec = np.where(tri, omb * a ** np.where(tri, m - k, 0.0), 0.0)
    vrow = a ** (m + 1.0)                          # carry weight, [1, 127]

    # block 0: x0 column is a^m (not (1-a)a^m); no carry; rows / d[m]
    W0 = np.where((k == 0) & tri, a**m, dec)
    W0 = np.concatenate([W0, np.zeros((1, L))], axis=0) / d[:L][None, :]
    # block 1: rows / d[127+m]; carry row undoes block 0's /d[126]
    W1 = np.concatenate([dec, vrow * d[L - 1]], axis=0) / d[L:2 * L][None, :]
    # blocks >= 2: debias is exactly 1.0 in f32
    W = np.concatenate([dec, vrow], axis=0)
    # final partial block (rem rows): K = rem + 1, padded to 128 partitions
    kl = np.arange(rem, dtype=np.float64)[:, None]
    ml = np.arange(rem, dtype=np.float64)[None, :]
    tril = (ml - kl) >= 0
    decl = np.where(tril, omb * a ** np.where(tril, ml - kl, 0.0), 0.0)
    Wlast = np.concatenate(
        [decl, a ** (ml + 1.0), np.zeros((L - rem, rem))], axis=0)

    w = np.concatenate([W0, W1, W, Wlast], axis=1)
    assert w.shape == (128, 3 * L + rem)
    return np.ascontiguousarray(w.astype(np.float32))


def build_program(bpc: int = BPC, t_len: int = T, chunk: int = 4):
    """One core's program: EMA over `bpc` independent [t_len, C] sequences,
    interleaved round-robin so each carry chain has `bpc` blocks of slack."""
    nfull, rem = _plan(t_len)
    nchunk = nfull // chunk
    assert nchunk * chunk == nfull and rem > 0
    assert bpc <= 4

    nc = bacc.Bacc("TRN2", target_bir_lowering=False, debug=False)
    x = nc.dram_tensor("x", [bpc * t_len, C], F32, kind="ExternalInput").ap()
    w = nc.dram_tensor("w", [128, 3 * L + rem], F32,
                       kind="ExternalInput").ap()
    y = nc.dram_tensor("y", [bpc * t_len, C], F32, kind="ExternalOutput").ap()

    with TileContext(nc) as tc:
        with (
            tc.tile_pool(name="wpool", bufs=1) as wpool,
            tc.tile_pool(name="xpool", bufs=2 * bpc + 1) as xpool,
            tc.tile_pool(name="ypool", bufs=2 * bpc + 1) as ypool,
            tc.tile_pool(name="psum", bufs=8, space="PSUM") as ppool,
        ):
            wt = wpool.tile([128, 3 * L + rem], F32)
            nc.sync.dma_start(out=wt[:, :], in_=w[:, :])
            W0w = wt[:, 0 * L:1 * L]
            W1w = wt[:, 1 * L:2 * L]
            Ww = wt[:, 2 * L:3 * L]
            Wlast = wt[0:rem + 1, 3 * L:3 * L + rem]

            nblk = nfull + 1          # per-sequence blocks incl. partial
            xt: dict = {}
            yt: dict = {}
            # (dram_row, width, tile, free_off, carry_part) per (b, block)
            eng_i = 0

            def alloc_chunk(ch):
                """Allocate + DMA the x tiles for chunk `ch` of each batch,
                returning {b: tile}.  Chunk ch covers blocks
                ch*chunk..ch*chunk+chunk-1 (127 rows each) or the final
                partial block when ch == nchunk."""
                tiles = {}
                for b in range(bpc):
                    if ch < nchunk:
                        r0 = b * t_len + ch * chunk * L
                        tl = xpool.tile([128, chunk * C], F32, tag="xt",
                                        name=f"xt_{ch}_{b}")
                        if ch == 0:
                            # block 0 has no carry: zero its state slot
                            # (partition 127; compute ops must start 32-
                            # aligned, so clear [96:128) BEFORE the x DMA
                            # overwrites rows 96..126 with real data)
                            nc.gpsimd.memset(tl[96:128, 0:C], 0.0)
                            nc.sync.dma_start(out=tl[0:L, 0:C],
                                              in_=x[r0:r0 + L, :])
                            if chunk > 1:
                                nc.sync.dma_start(
                                    out=tl[0:L, C:].rearrange(
                                        "p (n c) -> p n c", c=C),
                                    in_=x[r0 + L:r0 + chunk * L, :].rearrange(
                                        "(n p) c -> p n c", p=L),
                                )
                        else:
                            nc.sync.dma_start(
                                out=tl[0:L, :].rearrange(
                                    "p (n c) -> p n c", c=C),
                                in_=x[r0:r0 + chunk * L, :].rearrange(
                                    "(n p) c -> p n c", p=L),
                            )
                    else:
                        r0 = b * t_len + nfull * L
                        tl = xpool.tile([128, C], F32, tag="xt",
                                        name=f"xt_last_{b}")
                        nc.sync.dma_start(out=tl[0:rem, :],
                                          in_=x[r0:r0 + rem, :])
                    tiles[b] = tl
                return tiles

            xt = alloc_chunk(0)
            for ch in range(nchunk + 1):
                last_chunk = ch == nchunk
                blocks = 1 if last_chunk else chunk
                for b in range(bpc):
                    yt[b] = ypool.tile(
                        [128, C if last_chunk else chunk * C], F32,
                        tag="yt", name=f"yt_{ch}_{b}")
                nxt = None
                for j in range(blocks):
                    r = ch * chunk + j
                    if j == 0 and not last_chunk:
                        nxt = alloc_chunk(ch + 1)   # prefetch next chunk
                    ps = {}
                    for b in range(bpc):
                        if last_chunk:
                            ps[b] = ppool.tile([rem, C], F32, tag="ps",
                                               name=f"ps_{ch}_{j}_{b}")
                            nc.tensor.matmul(ps[b][:, :], Wlast,
                                             xt[b][0:rem + 1, 0:C],
                                             start=True, stop=True)
                        else:
                            ps[b] = ppool.tile([L, C], F32, tag="ps",
                                               name=f"ps_{ch}_{j}_{b}")
                            cw = W0w if r == 0 else (W1w if r == 1 else Ww)
                            nc.tensor.matmul(ps[b][:, :], cw,
                                             xt[b][:, j * C:(j + 1) * C],
                                             start=True, stop=True)
                    for b in range(bpc):
                        rows = rem if last_chunk else L
                        if r < nblk - 1:
                            # high rows (96..126, incl. the carry row 126)
                            # first, as a small 32-aligned copy, so the
                            # carry DMA never waits on the full-tile copy
                            nc.vector.tensor_copy(
                                out=yt[b][96:L, j * C:(j + 1) * C],
                                in_=ps[b][96:L, :])
                            # carry -> state slot of the next block's rhs
                            if j + 1 < blocks:
                                tgt = xt[b][L:L + 1, (j + 1) * C:(j + 2) * C]
                            elif ch + 1 == nchunk:
                                tgt = nxt[b][rem:rem + 1, 0:C]
                            else:
                                tgt = nxt[b][L:L + 1, 0:C]
                            # scalar (ACT) HWDGE ring: keeps the latency-
                            # critical carry hop off the ring that carries
                            # the prefetched 1MB x loads (FIFO HOL)
                            nc.scalar.dma_start(
                                out=tgt,
                                in_=yt[b][L - 1:L, j * C:(j + 1) * C])
                            dst = yt[b][0:96, j * C:(j + 1) * C]
                            src = ps[b][0:96, :]
                        else:
                            dst = yt[b][0:rows, j * C:(j + 1) * C]
                            src = ps[b][0:rows, :]
                        if eng_i % 2 == 0:
                            nc.vector.tensor_copy(out=dst, in_=src)
                        else:
                            nc.scalar.copy(dst, src)
                        eng_i += 1
                if nxt is not None:
                    xt = nxt
                for b in range(bpc):
                    if last_chunk:
                        r0 = b * t_len + nfull * L
                        nc.gpsimd.dma_start(out=y[r0:r0 + rem, :],
                                            in_=yt[b][0:rem, :])
                    else:
                        r0 = b * t_len + ch * chunk * L
                        nc.gpsimd.dma_start(
                            out=y[r0:r0 + chunk * L, :].rearrange(
                                "(n p) c -> p n c", p=L),
                            in_=yt[b][0:L, :].rearrange(
                                "p (n c) -> p n c", c=C),
                        )
    nc.compile()
    return nc


_CACHE: dict = {}


def _get_program():
    if "nc" not in _CACHE:
        _CACHE["nc"] = build_program()
        _CACHE["w"] = _build_weights()
    return _CACHE["nc"], _CACHE["w"]


def _run(x: np.ndarray, trace: bool = False):
    nc, w = _get_program()
    in_maps = [
        {
            "x": np.ascontiguousarray(
                x[k * BPC:(k + 1) * BPC].reshape(BPC * T, C)),
            "w": w,
        }
        for k in range(NCORES)
    ]
    res = bass_utils.run_bass_kernel_spmd(
        nc, in_maps, core_ids=list(range(NCORES)), trace=trace)
    y = np.concatenate(
        [r["y"].reshape(BPC, T, C) for r in res.results], axis=0)
    return y, res


def kernel(x) -> np.ndarray:
    x = np.asarray(x, dtype=np.float32)
    assert x.shape == (B, T, C), x.shape
    y, _ = _run(x, trace=False)
    return y


# revision 15
# speedup vs baseline: 1.7656x; 1.6903x over previous
"""Debiased EMA (nn_DebiasedEMA) Trainium2 Bass kernel.

x: [B=32, T=4096, C=512] f32.
    y_t = a*y_{t-1} + (1-a)*x_t  (y_0 = x_0), a = f32(0.9)
    out_t = y_t / max(1 - a^(t+1), 1e-6)

Formulation (exact, no window truncation): split T into 127-row blocks in
the natural [T, C] layout.  For block i with local row m (t = 127i + m),

    y_t = sum_{k=0..126} (1-a) a^(m-k) [k<=m] * x_block_i[k]
        + a^(m+1) * carry_{i-1}

with carry_{i-1} = y_{127i-1}, the last EMA row of the previous block.
Both terms fold into a SINGLE K=128 fp32 matmul per block: the rhs tile
holds the 127 current x rows on partitions 0..126 and the carry vector on
partition 127; the lhsT weight matrix holds the triangular decay profile
in rows 0..126 and v[m] = a^(m+1) in row 127.  PE cost: one fp32 matmul
(~0.85us) per 127 output rows — half of the sliding-window formulation.

The debias divisor is exactly 1.0 in fp32 for t >= 165, so it is folded
into the block-0/1 weight matrices (block 0 also carries the y_0 = x_0
initial condition in its k=0 column, and zero in the carry row); block 1's
carry row also undoes block 0's row-126 debias.

The carry chain is a 2KB SBUF->SBUF DMA per block: output row 126 (copied
out of PSUM first, as a tiny row copy) -> partition 127 of the next
block's rhs slice.  The 4 sequences per core are interleaved round-robin,
so each chain step has ~4 matmuls of slack.

T = 4096 = 32*127 + 32: the final partial block uses a K=33 matmul
(32 x rows + carry on partition 32).

Sharding: batch-parallel, 4 sequences per NeuronCore, no communication.
"""

import dataclasses
import sys

for _p in ("/opt/trn_rl_repo", "/opt/pypackages"):
    if _p not in sys.path:
        sys.path.insert(0, _p)

import numpy as np

import concourse.bacc as bacc
import concourse.mybir as mybir
from concourse import bass_utils
from concourse.tile import TileContext

B, T, C = 32, 4096, 512
NCORES = 8
BPC = B // NCORES        # sequences per core
L = 127                  # time-block length (partition 127 carries state)
ALPHA = 0.9
DENOM_MIN = 1e-6

F32 = mybir.dt.float32


def _plan(t_len: int):
    """Number of full 127-row blocks and the size of the final partial."""
    nfull = t_len // L
    rem = t_len - nfull * L
    return nfull, rem


def _build_weights(t_len: int = T) -> np.ndarray:
    """lhsT weight matrices, packed [128, 3*127 + rem] f32.

    Columns: [W0 (block 0) | W1 (block 1) | W (blocks >= 2) | Wlast (rem)].
    Each W* is [128, 127]: rows k=0..126 hold the triangular decay
    (1-a)a^(m-k) (block 0: a^m in the k=0 column instead), row k=127 holds
    the carry weight a^(m+1) (block 0: zero).  Debias folds: block 0 rows
    /d[m], block 1 rows /d[127+m] with carry row also * d[126].
    Wlast is [33, rem] padded to [128, rem]: 32 x rows + carry row at k=32.
    Built in float64 from the f32-rounded alpha, rounded once to f32.
    """
    a = float(np.float32(ALPHA))     # f32 value of clip(0.9) as f64
    omb = 1.0 - a                    # exact (Sterbenz), matches f32 1-a
    nfull, rem = _plan(t_len)
    t = np.arange(2 * L + 2, dtype=np.float64)
    d = np.maximum(1.0 - a ** (t + 1.0), DENOM_MIN)

    k = np.arange(L, dtype=np.float64)[:, None]   # input row 0..126
    m = np.arange(L, dtype=np.float64)[None, :]   # output row 0..126
    tri = (m - k) >= 0
    dec = np.where(tri, omb * a ** np.where(tri, m - k, 0.0), 0.0)
    vrow = a ** (m + 1.0)                          # carry weight, [1, 127]

    # block 0: x0 column is a^m (not (1-a)a^m); no carry; rows / d[m]
    W0 = np.where((k == 0) & tri, a**m, dec)
    W0 = np.concatenate([W0, np.zeros((1, L))], axis=0) / d[:L][None, :]
    # block 1: rows / d[127+m]; carry row undoes block 0's /d[126]
    W1 = np.concatenate([dec, vrow * d[L - 1]], axis=0) / d[L:2 * L][None, :]
    # blocks >= 2: debias is exactly 1.0 in f32
    W = np.concatenate([dec, vrow], axis=0)
    # final partial block (rem rows): K = rem + 1, padded to 128 partitions
    kl = np.arange(rem, dtype=np.float64)[:, None]
    ml = np.arange(rem, dtype=np.float64)[None, :]
    tril = (ml - kl) >= 0
    decl = np.where(tril, omb * a ** np.where(tril, ml - kl, 0.0), 0.0)
    Wlast = np.concatenate(
        [decl, a ** (ml + 1.0), np.zeros((L - rem, rem))], axis=0)

    w = np.concatenate([W0, W1, W, Wlast], axis=1)
    assert w.shape == (128, 3 * L + rem)
    return np.ascontiguousarray(w.astype(np.float32))


def build_program(bpc: int = BPC, t_len: int = T, chunk: int = 4):
    """One core's program: EMA over `bpc` independent [t_len, C] sequences,
    interleaved round-robin so each carry chain has `bpc` blocks of slack."""
    nfull, rem = _plan(t_len)
    nchunk = nfull // chunk
    assert nchunk * chunk == nfull and rem > 0
    assert bpc <= 4

    nc = bacc.Bacc("TRN2", target_bir_lowering=False, debug=False)
    x = nc.dram_tensor("x", [bpc * t_len, C], F32, kind="ExternalInput").ap()
    w = nc.dram_tensor("w", [128, 3 * L + rem], F32,
                       kind="ExternalInput").ap()
    y = nc.dram_tensor("y", [bpc * t_len, C], F32, kind="ExternalOutput").ap()

    with TileContext(nc) as tc:
        with (
            tc.tile_pool(name="wpool", bufs=1) as wpool,
            tc.tile_pool(name="xpool", bufs=2 * bpc + 1) as xpool,
            tc.tile_pool(name="ypool", bufs=2 * bpc + 1) as ypool,
            tc.tile_pool(name="psum", bufs=8, space="PSUM") as ppool,
        ):
            wt = wpool.tile([128, 3 * L + rem], F32)
            nc.sync.dma_start(out=wt[:, :], in_=w[:, :])
            W0w = wt[:, 0 * L:1 * L]
            W1w = wt[:, 1 * L:2 * L]
            Ww = wt[:, 2 * L:3 * L]
            Wlast = wt[0:rem + 1, 3 * L:3 * L + rem]

            nblk = nfull + 1          # per-sequence blocks incl. partial
            xt: dict = {}
            yt: dict = {}
            # (dram_row, width, tile, free_off, carry_part) per (b, block)
            eng_i = 0

            def overlap_in(row0, nwin):
                """DRAM AP reading rows row0 + 127*n + p (p<128, n<nwin):
                128-partition descriptors (the fast HWDGE shape) with a
                1-row overlap between consecutive windows; the overlap row
                (partition 127) is later overwritten by the carry DMA."""
                return dataclasses.replace(
                    x, offset=row0 * C,
                    ap=[[C, 128], [L * C, nwin], [1, C]])

            def alloc_chunk(ch):
                """Allocate + DMA the x tiles for chunk `ch` of each batch,
                returning {b: tile}.  Chunk ch covers blocks
                ch*chunk..ch*chunk+chunk-1 (127 rows each) or the final
                partial block when ch == nchunk.  Block j's rhs slice gets
                x rows on partitions 0..126 plus a junk overlap row on
                partition 127 that the carry DMA (weight row: zero for
                block 0) replaces."""
                tiles = {}
                for b in range(bpc):
                    if ch < nchunk:
                        r0 = b * t_len + ch * chunk * L
                        tl = xpool.tile([128, chunk * C], F32, tag="xt",
                                        name=f"xt_{ch}_{b}")
                        if ch == 0:
                            # split the first window for an early start
                            nc.sync.dma_start(out=tl[:, 0:C],
                                              in_=x[r0:r0 + 128, :])
                            if chunk > 1:
                                nc.sync.dma_start(
                                    out=tl[:, C:].rearrange(
                                        "p (n c) -> p n c", c=C),
                                    in_=overlap_in(r0 + L, chunk - 1),
                                )
                        else:
                            nc.sync.dma_start(
                                out=tl[:, :].rearrange(
                                    "p (n c) -> p n c", c=C),
                                in_=overlap_in(r0, chunk),
                            )
                    else:
                        r0 = b * t_len + nfull * L
                        tl = xpool.tile([128, C], F32, tag="xt",
                                        name=f"xt_last_{b}")
                        # odd partition counts are slow on HWDGE; use SWDGE
                        nc.gpsimd.dma_start(out=tl[0:rem, :],
                                            in_=x[r0:r0 + rem, :])
                    tiles[b] = tl
                return tiles

            xt = alloc_chunk(0)
            for ch in range(nchunk + 1):
                last_chunk = ch == nchunk
                blocks = 1 if last_chunk else chunk
                for b in range(bpc):
                    yt[b] = ypool.tile(
                        [128, C if last_chunk else chunk * C], F32,
                        tag="yt", name=f"yt_{ch}_{b}")
                nxt = None
                for j in range(blocks):
                    r = ch * chunk + j
                    if j == 0 and not last_chunk:
                        nxt = alloc_chunk(ch + 1)   # prefetch next chunk
                    ps = {}
                    for b in range(bpc):
                        if last_chunk:
                            ps[b] = ppool.tile([rem, C], F32, tag="ps",
                                               name=f"ps_{ch}_{j}_{b}")
                            nc.tensor.matmul(ps[b][:, :], Wlast,
                                             xt[b][0:rem + 1, 0:C],
                                             start=True, stop=True)
                        else:
                            ps[b] = ppool.tile([L, C], F32, tag="ps",
                                               name=f"ps_{ch}_{j}_{b}")
                            cw = W0w if r == 0 else (W1w if r == 1 else Ww)
                            nc.tensor.matmul(ps[b][:, :], cw,
                                             xt[b][:, j * C:(j + 1) * C],
                                             start=True, stop=True)
                    for b in range(bpc):
                        rows = rem if last_chunk else L
                        if r < nblk - 1:
                            # high rows (96..126, incl. the carry row 126)
                            # first, as a small 32-aligned copy, so the
                            # carry DMA never waits on the full-tile copy
                            nc.vector.tensor_copy(
                                out=yt[b][96:L, j * C:(j + 1) * C],
                                in_=ps[b][96:L, :])
                            # carry -> state slot of the next block's rhs
                            if j + 1 < blocks:
                                tgt = xt[b][L:L + 1, (j + 1) * C:(j + 2) * C]
                            elif ch + 1 == nchunk:
                                tgt = nxt[b][rem:rem + 1, 0:C]
                            else:
                                tgt = nxt[b][L:L + 1, 0:C]
                            # scalar (ACT) HWDGE ring: keeps the latency-
                            # critical carry hop off the ring that carries
                            # the prefetched 1MB x loads (FIFO HOL)
                            nc.scalar.dma_start(
                                out=tgt,
                                in_=yt[b][L - 1:L, j * C:(j + 1) * C])
                            dst = yt[b][0:96, j * C:(j + 1) * C]
                            src = ps[b][0:96, :]
                        else:
                            dst = yt[b][0:rows, j * C:(j + 1) * C]
                            src = ps[b][0:rows, :]
                        if eng_i % 2 == 0:
                            nc.vector.tensor_copy(out=dst, in_=src)
                        else:
                            nc.scalar.copy(dst, src)
                        eng_i += 1
                if nxt is not None:
                    xt = nxt
                for b in range(bpc):
                    if last_chunk:
                        r0 = b * t_len + nfull * L
                        nc.gpsimd.dma_start(out=y[r0:r0 + rem, :],
                                            in_=yt[b][0:rem, :])
                    else:
                        r0 = b * t_len + ch * chunk * L
                        nc.gpsimd.dma_start(
                            out=y[r0:r0 + chunk * L, :].rearrange(
                                "(n p) c -> p n c", p=L),
                            in_=yt[b][0:L, :].rearrange(
                                "p (n c) -> p n c", c=C),
                        )
    nc.compile()
    return nc


_CACHE: dict = {}


def _get_program():
    if "nc" not in _CACHE:
        _CACHE["nc"] = build_program()
        _CACHE["w"] = _build_weights()
    return _CACHE["nc"], _CACHE["w"]


def _run(x: np.ndarray, trace: bool = False):
    nc, w = _get_program()
    in_maps = [
        {
            "x": np.ascontiguousarray(
                x[k * BPC:(k + 1) * BPC].reshape(BPC * T, C)),
            "w": w,
        }
        for k in range(NCORES)
    ]
    res = bass_utils.run_bass_kernel_spmd(
        nc, in_maps, core_ids=list(range(NCORES)), trace=trace)
    y = np.concatenate(
        [r["y"].reshape(BPC, T, C) for r in res.results], axis=0)
    return y, res


def kernel(x) -> np.ndarray:
    x = np.asarray(x, dtype=np.float32)
    assert x.shape == (B, T, C), x.shape
    y, _ = _run(x, trace=False)
    return y


# revision 16
# speedup vs baseline: 2.8622x; 1.6211x over previous
"""Debiased EMA (nn_DebiasedEMA) Trainium2 Bass kernel.

x: [B=32, T=4096, C=512] f32.
    y_t = a*y_{t-1} + (1-a)*x_t  (y_0 = x_0), a = f32(0.9)
    out_t = y_t / max(1 - a^(t+1), 1e-6)

Formulation (exact, no window truncation): split T into 127-row blocks in
the natural [T, C] layout.  For block i with local row m (t = 127i + m),

    y_t = sum_{k=0..126} (1-a) a^(m-k) [k<=m] * x_block_i[k]
        + a^(m+1) * carry_{i-1}

with carry_{i-1} = y_{127i-1}, the last EMA row of the previous block.
Both terms fold into a SINGLE K=128 fp32 matmul per block: the rhs tile
holds the 127 current x rows on partitions 0..126 and the carry vector on
partition 127; the lhsT weight matrix holds the triangular decay profile
in rows 0..126 and v[m] = a^(m+1) in row 127.  PE cost: one fp32 matmul
(~0.85us) per 127 output rows — half of the sliding-window formulation.

The debias divisor is exactly 1.0 in fp32 for t >= 165, so it is folded
into the block-0/1 weight matrices (block 0 also carries the y_0 = x_0
initial condition in its k=0 column, and zero in the carry row); block 1's
carry row also undoes block 0's row-126 debias.

The carry chain is a 2KB SBUF->SBUF DMA per block: output row 126 (copied
out of PSUM first, as a tiny row copy) -> partition 127 of the next
block's rhs slice.  The 4 sequences per core are interleaved round-robin,
so each chain step has ~4 matmuls of slack.

T = 4096 = 32*127 + 32: the final partial block uses a K=33 matmul
(32 x rows + carry on partition 32).

Sharding: batch-parallel, 4 sequences per NeuronCore, no communication.
"""

import dataclasses
import sys

for _p in ("/opt/trn_rl_repo", "/opt/pypackages"):
    if _p not in sys.path:
        sys.path.insert(0, _p)

import numpy as np

import concourse.bacc as bacc
import concourse.mybir as mybir
from concourse import bass_utils
from concourse.tile import TileContext

B, T, C = 32, 4096, 512
NCORES = 8
BPC = B // NCORES        # sequences per core
L = 127                  # time-block length (partition 127 carries state)
ALPHA = 0.9
DENOM_MIN = 1e-6

F32 = mybir.dt.float32


def _plan(t_len: int):
    """Number of full 127-row blocks and the size of the final partial."""
    nfull = t_len // L
    rem = t_len - nfull * L
    return nfull, rem


def _build_weights(t_len: int = T) -> np.ndarray:
    """lhsT weight matrices, packed [128, 3*127 + rem] f32.

    Columns: [W0 (block 0) | W1 (block 1) | W (blocks >= 2) | Wlast (rem)].
    Each W* is [128, 127]: rows k=0..126 hold the triangular decay
    (1-a)a^(m-k) (block 0: a^m in the k=0 column instead), row k=127 holds
    the carry weight a^(m+1) (block 0: zero).  Debias folds: block 0 rows
    /d[m], block 1 rows /d[127+m] with carry row also * d[126].
    Wlast is [33, rem] padded to [128, rem]: 32 x rows + carry row at k=32.
    Built in float64 from the f32-rounded alpha, rounded once to f32.
    """
    a = float(np.float32(ALPHA))     # f32 value of clip(0.9) as f64
    omb = 1.0 - a                    # exact (Sterbenz), matches f32 1-a
    nfull, rem = _plan(t_len)
    t = np.arange(2 * L + 2, dtype=np.float64)
    d = np.maximum(1.0 - a ** (t + 1.0), DENOM_MIN)

    k = np.arange(L, dtype=np.float64)[:, None]   # input row 0..126
    m = np.arange(L, dtype=np.float64)[None, :]   # output row 0..126
    tri = (m - k) >= 0
    dec = np.where(tri, omb * a ** np.where(tri, m - k, 0.0), 0.0)
    vrow = a ** (m + 1.0)                          # carry weight, [1, 127]

    # block 0: x0 column is a^m (not (1-a)a^m); no carry; rows / d[m]
    W0 = np.where((k == 0) & tri, a**m, dec)
    W0 = np.concatenate([W0, np.zeros((1, L))], axis=0) / d[:L][None, :]
    # block 1: rows / d[127+m]; carry row undoes block 0's /d[126]
    W1 = np.concatenate([dec, vrow * d[L - 1]], axis=0) / d[L:2 * L][None, :]
    # blocks >= 2: debias is exactly 1.0 in f32
    W = np.concatenate([dec, vrow], axis=0)
    # final partial block (rem rows): K = rem + 1, padded to 128 partitions
    kl = np.arange(rem, dtype=np.float64)[:, None]
    ml = np.arange(rem, dtype=np.float64)[None, :]
    tril = (ml - kl) >= 0
    decl = np.where(tril, omb * a ** np.where(tril, ml - kl, 0.0), 0.0)
    Wlast = np.concatenate(
        [decl, a ** (ml + 1.0), np.zeros((L - rem, rem))], axis=0)

    w = np.concatenate([W0, W1, W, Wlast], axis=1)
    assert w.shape == (128, 3 * L + rem)
    return np.ascontiguousarray(w.astype(np.float32))


def build_program(bpc: int = BPC, t_len: int = T, chunk: int = 4):
    """One core's program: EMA over `bpc` independent [t_len, C] sequences,
    interleaved round-robin so each carry chain has `bpc` blocks of slack."""
    nfull, rem = _plan(t_len)
    nchunk = nfull // chunk
    assert nchunk * chunk == nfull and rem > 0
    assert bpc <= 4

    nc = bacc.Bacc("TRN2", target_bir_lowering=False, debug=False)
    x = nc.dram_tensor("x", [bpc * t_len, C], F32, kind="ExternalInput").ap()
    w = nc.dram_tensor("w", [128, 3 * L + rem], F32,
                       kind="ExternalInput").ap()
    y = nc.dram_tensor("y", [bpc * t_len, C], F32, kind="ExternalOutput").ap()

    with TileContext(nc) as tc:
        with (
            tc.tile_pool(name="wpool", bufs=1) as wpool,
            tc.tile_pool(name="xpool", bufs=2 * bpc + 1) as xpool,
            tc.tile_pool(name="ypool", bufs=3 * bpc) as ypool,
            tc.tile_pool(name="psum", bufs=8, space="PSUM") as ppool,
        ):
            wt = wpool.tile([128, 3 * L + rem], F32)
            nc.sync.dma_start(out=wt[:, :], in_=w[:, :])
            W0w = wt[:, 0 * L:1 * L]
            W1w = wt[:, 1 * L:2 * L]
            Ww = wt[:, 2 * L:3 * L]
            Wlast = wt[0:rem + 1, 3 * L:3 * L + rem]

            nblk = nfull + 1          # per-sequence blocks incl. partial
            xt: dict = {}
            yt: dict = {}
            # (dram_row, width, tile, free_off, carry_part) per (b, block)
            eng_i = 0

            def overlap_in(row0, nwin):
                """DRAM AP reading rows row0 + 127*n + p (p<128, n<nwin):
                128-partition descriptors (the fast HWDGE shape) with a
                1-row overlap between consecutive windows; the overlap row
                (partition 127) is later overwritten by the carry DMA."""
                return dataclasses.replace(
                    x, offset=row0 * C,
                    ap=[[C, 128], [L * C, nwin], [1, C]])

            def alloc_chunk(ch):
                """Allocate + DMA the x tiles for chunk `ch` of each batch,
                returning {b: tile}.  Chunk ch covers blocks
                ch*chunk..ch*chunk+chunk-1 (127 rows each) or the final
                partial block when ch == nchunk.  Block j's rhs slice gets
                x rows on partitions 0..126 plus a junk overlap row on
                partition 127 that the carry DMA (weight row: zero for
                block 0) replaces."""
                tiles = {}
                for b in range(bpc):
                    if ch < nchunk:
                        r0 = b * t_len + ch * chunk * L
                        tl = xpool.tile([128, chunk * C], F32, tag="xt",
                                        name=f"xt_{ch}_{b}")
                        if ch == 0:
                            # split the first window for an early start
                            nc.sync.dma_start(out=tl[:, 0:C],
                                              in_=x[r0:r0 + 128, :])
                            if chunk > 1:
                                nc.sync.dma_start(
                                    out=tl[:, C:].rearrange(
                                        "p (n c) -> p n c", c=C),
                                    in_=overlap_in(r0 + L, chunk - 1),
                                )
                        else:
                            nc.sync.dma_start(
                                out=tl[:, :].rearrange(
                                    "p (n c) -> p n c", c=C),
                                in_=overlap_in(r0, chunk),
                            )
                    else:
                        r0 = b * t_len + nfull * L
                        tl = xpool.tile([128, C], F32, tag="xt",
                                        name=f"xt_last_{b}")
                        # odd partition counts are slow on HWDGE; use SWDGE
                        nc.gpsimd.dma_start(out=tl[0:rem, :],
                                            in_=x[r0:r0 + rem, :])
                    tiles[b] = tl
                return tiles

            xt = alloc_chunk(0)
            for ch in range(nchunk + 1):
                last_chunk = ch == nchunk
                blocks = 1 if last_chunk else chunk
                for b in range(bpc):
                    yt[b] = ypool.tile(
                        [128, C if last_chunk else chunk * C], F32,
                        tag="yt", name=f"yt_{ch}_{b}")
                nxt = None
                for j in range(blocks):
                    r = ch * chunk + j
                    if j == 0 and not last_chunk:
                        nxt = alloc_chunk(ch + 1)   # prefetch next chunk
                    ps = {}
                    for b in range(bpc):
                        if last_chunk:
                            ps[b] = ppool.tile([rem, C], F32, tag="ps",
                                               name=f"ps_{ch}_{j}_{b}")
                            nc.tensor.matmul(ps[b][:, :], Wlast,
                                             xt[b][0:rem + 1, 0:C],
                                             start=True, stop=True)
                        else:
                            ps[b] = ppool.tile([L, C], F32, tag="ps",
                                               name=f"ps_{ch}_{j}_{b}")
                            cw = W0w if r == 0 else (W1w if r == 1 else Ww)
                            nc.tensor.matmul(ps[b][:, :], cw,
                                             xt[b][:, j * C:(j + 1) * C],
                                             start=True, stop=True)
                    for b in range(bpc):
                        rows = rem if last_chunk else L
                        if r < nblk - 1:
                            # high rows (96..126, incl. the carry row 126)
                            # first, as a small 32-aligned copy, so the
                            # carry DMA never waits on the full-tile copy
                            nc.vector.tensor_copy(
                                out=yt[b][96:L, j * C:(j + 1) * C],
                                in_=ps[b][96:L, :])
                            # carry -> state slot of the next block's rhs
                            if j + 1 < blocks:
                                tgt = xt[b][L:L + 1, (j + 1) * C:(j + 2) * C]
                            elif ch + 1 == nchunk:
                                tgt = nxt[b][rem:rem + 1, 0:C]
                            else:
                                tgt = nxt[b][L:L + 1, 0:C]
                            # scalar (ACT) HWDGE ring: keeps the latency-
                            # critical carry hop off the ring that carries
                            # the prefetched 1MB x loads (FIFO HOL)
                            nc.scalar.dma_start(
                                out=tgt,
                                in_=yt[b][L - 1:L, j * C:(j + 1) * C])
                            dst = yt[b][0:96, j * C:(j + 1) * C]
                            src = ps[b][0:96, :]
                        else:
                            dst = yt[b][0:rows, j * C:(j + 1) * C]
                            src = ps[b][0:rows, :]
                        if eng_i % 2 == 0:
                            nc.vector.tensor_copy(out=dst, in_=src)
                        else:
                            nc.scalar.copy(dst, src)
                        eng_i += 1
                if nxt is not None:
                    xt = nxt
                for b in range(bpc):
                    if last_chunk:
                        r0 = b * t_len + nfull * L
                        nc.gpsimd.dma_start(out=y[r0:r0 + rem, :],
                                            in_=yt[b][0:rem, :])
                    else:
                        # halves, so draining starts after 2 blocks' copies
                        h = chunk // 2
                        for half in range(2):
                            r0 = b * t_len + (ch * chunk + half * h) * L
                            fo = half * h * C
                            nc.gpsimd.dma_start(
                                out=y[r0:r0 + h * L, :].rearrange(
                                    "(n p) c -> p n c", p=L),
                                in_=yt[b][0:L, fo:fo + h * C].rearrange(
                                    "p (n c) -> p n c", c=C),
                            )
    nc.compile()
    return nc


_CACHE: dict = {}


def _get_program():
    if "nc" not in _CACHE:
        _CACHE["nc"] = build_program()
        _CACHE["w"] = _build_weights()
    return _CACHE["nc"], _CACHE["w"]


def _run(x: np.ndarray, trace: bool = False):
    nc, w = _get_program()
    in_maps = [
        {
            "x": np.ascontiguousarray(
                x[k * BPC:(k + 1) * BPC].reshape(BPC * T, C)),
            "w": w,
        }
        for k in range(NCORES)
    ]
    res = bass_utils.run_bass_kernel_spmd(
        nc, in_maps, core_ids=list(range(NCORES)), trace=trace)
    y = np.concatenate(
        [r["y"].reshape(BPC, T, C) for r in res.results], axis=0)
    return y, res


def kernel(x) -> np.ndarray:
    x = np.asarray(x, dtype=np.float32)
    assert x.shape == (B, T, C), x.shape
    y, _ = _run(x, trace=False)
    return y


# revision 17
# speedup vs baseline: 7.1128x; 2.4851x over previous
"""Fallback kernel (v4): sliding-window two-matmul formulation, 245.9us.

out_block_i = P.T @ x_block_{i-1} + C.T @ x_block_i in natural [T, C]
layout, 128-row blocks, both matmuls fp32; debias folded into block-0/1
weights.  Exact to fp32 except an a^256 ~ 2e-12 window truncation.
"""

import sys

for _p in ("/opt/trn_rl_repo", "/opt/pypackages"):
    if _p not in sys.path:
        sys.path.insert(0, _p)

import numpy as np

import concourse.bacc as bacc
import concourse.mybir as mybir
from concourse import bass_utils
from concourse.tile import TileContext

B, T, C = 32, 4096, 512
NCORES = 8
BPC = B // NCORES
L = 128
ALPHA = 0.9
DENOM_MIN = 1e-6

F32 = mybir.dt.float32


def _build_weights() -> np.ndarray:
    a = float(np.float32(ALPHA))
    omb = 1.0 - a
    k = np.arange(L, dtype=np.float64)[:, None]
    m = np.arange(L, dtype=np.float64)[None, :]
    tri = (m - k) >= 0
    t = np.arange(2 * L, dtype=np.float64)
    d = np.maximum(1.0 - a ** (t + 1.0), DENOM_MIN)
    dec = np.where(tri, a ** np.where(tri, m - k, 0.0), 0.0)
    x0col = (k == 0)
    A0 = np.where(tri, np.where(x0col, a**m, omb * dec), 0.0) / d[:L][None, :]
    P1 = np.where(x0col, a ** (128.0 + m), omb * a ** (128.0 + m - k)) \
        / d[L:][None, :]
    C1 = omb * dec / d[L:][None, :]
    P = omb * a ** (128.0 + m - k)
    Cm = omb * dec
    w = np.concatenate([A0, P1, C1, P, Cm], axis=1)
    return np.ascontiguousarray(w.astype(np.float32))


def build_program(bpc: int = BPC, t_len: int = T, chunk: int = 8):
    nblk = t_len // L
    nchunk = nblk // chunk
    assert nblk * L == t_len and nchunk * chunk == nblk

    nc = bacc.Bacc("TRN2", target_bir_lowering=False, debug=False)
    x = nc.dram_tensor("x", [bpc * t_len, C], F32, kind="ExternalInput").ap()
    w = nc.dram_tensor("w", [L, 5 * L], F32, kind="ExternalInput").ap()
    y = nc.dram_tensor("y", [bpc * t_len, C], F32, kind="ExternalOutput").ap()

    with TileContext(nc) as tc:
        with (
            tc.tile_pool(name="wpool", bufs=1) as wpool,
            tc.tile_pool(name="xpool", bufs=4) as xpool,
            tc.tile_pool(name="ypool", bufs=4) as ypool,
            tc.tile_pool(name="psum", bufs=8, space="PSUM") as ppool,
        ):
            wt = wpool.tile([L, 5 * L], F32)
            nc.sync.dma_start(out=wt[:, :], in_=w[:, :])
            A0w = wt[:, 0 * L:1 * L]
            P1w = wt[:, 1 * L:2 * L]
            C1w = wt[:, 2 * L:3 * L]
            Pw = wt[:, 3 * L:4 * L]
            Cw = wt[:, 4 * L:5 * L]

            eng_i = 0
            for b in range(bpc):
                prev_chunk = None
                for ch in range(nchunk):
                    r0 = b * t_len + ch * chunk * L
                    xt = xpool.tile([L, chunk * C], F32, tag="xt",
                                    name=f"xt_{b}_{ch}")
                    if b == 0 and ch == 0:
                        nc.sync.dma_start(out=xt[:, 0:C], in_=x[r0:r0 + L, :])
                        nc.sync.dma_start(
                            out=xt[:, C:].rearrange("p (n c) -> p n c", c=C),
                            in_=x[r0 + L:r0 + chunk * L, :].rearrange(
                                "(n p) c -> p n c", p=L),
                        )
                    else:
                        nc.sync.dma_start(
                            out=xt[:, :].rearrange("p (n c) -> p n c", c=C),
                            in_=x[r0:r0 + chunk * L, :].rearrange(
                                "(n p) c -> p n c", p=L),
                        )
                    yt = ypool.tile([L, chunk * C], F32, tag="yt",
                                    name=f"yt_{b}_{ch}")
                    for j in range(chunk):
                        i = ch * chunk + j
                        cur = xt[:, j * C:(j + 1) * C]
                        ps = ppool.tile([L, C], F32, tag="ps",
                                        name=f"ps_{b}_{ch}_{j}")
                        if i == 0:
                            nc.tensor.matmul(ps[:, :], A0w, cur,
                                             start=True, stop=True)
                        else:
                            prev = (xt[:, (j - 1) * C:j * C] if j > 0
                                    else prev_chunk[:, (chunk - 1) * C:])
                            pw, cw = (P1w, C1w) if i == 1 else (Pw, Cw)
                            nc.tensor.matmul(ps[:, :], pw, prev,
                                             start=True, stop=False)
                            nc.tensor.matmul(ps[:, :], cw, cur,
                                             start=False, stop=True)
                        dst = yt[:, j * C:(j + 1) * C]
                        if eng_i % 2 == 0:
                            nc.vector.tensor_copy(out=dst, in_=ps[:, :])
                        else:
                            nc.scalar.copy(dst, ps[:, :])
                        eng_i += 1
                    h = chunk // 2
                    for half in range(2):
                        ro = r0 + half * h * L
                        fo = half * h * C
                        nc.gpsimd.dma_start(
                            out=y[ro:ro + h * L, :].rearrange(
                                "(n p) c -> p n c", p=L),
                            in_=yt[:, fo:fo + h * C].rearrange(
                                "p (n c) -> p n c", c=C),
                        )
                    prev_chunk = xt
    nc.compile()
    return nc


_CACHE: dict = {}


def _get_program():
    if "nc" not in _CACHE:
        _CACHE["nc"] = build_program()
        _CACHE["w"] = _build_weights()
    return _CACHE["nc"], _CACHE["w"]


def _run(x: np.ndarray, trace: bool = False):
    nc, w = _get_program()
    in_maps = [
        {
            "x": np.ascontiguousarray(
                x[k * BPC:(k + 1) * BPC].reshape(BPC * T, C)),
            "w": w,
        }
        for k in range(NCORES)
    ]
    res = bass_utils.run_bass_kernel_spmd(
        nc, in_maps, core_ids=list(range(NCORES)), trace=trace)
    y = np.concatenate(
        [r["y"].reshape(BPC, T, C) for r in res.results], axis=0)
    return y, res


def kernel(x) -> np.ndarray:
    x = np.asarray(x, dtype=np.float32)
    assert x.shape == (B, T, C), x.shape
    y, _ = _run(x, trace=False)
    return y
